# revision 21
# baseline (speedup 1.0000x reference)
"""AttributeImageCaptioner fused kernel for 8 trn2 NeuronCores.

Model (see reference):
  attr/word embedding gathers -> per-step LSTM (T=64, B=32, H=512) over
  inp = [word_emb(300) | image_feats(2048) | attr_mean(300)] -> masked h_seq
  -> vocab projection [512, 32000] (+argmax).

Distribution strategy:
  * The non-recurrent 84% of the LSTM contraction (inp @ W[:2648]) is
    batch-sharded: each core computes Zx for its 4 batch rows, then an
    AllGather shares the full Zx [2048 tok, 2048 gates].
  * The recurrence (h @ W[2648:3160] + gates) is replicated on all cores
    (per-step collectives are slower than the 3.4us/step of replicated work).
  * The vocab projection + argmax is vocab-sharded: each core holds a
    [512, 4000] slice of W_logits in SBUF and writes 1/8 of the logits.
  * Host combines per-core (max, argmax) candidates and concatenates logits.

Layout notes: everything downstream of Zx lives in "transposed" form
(h^T [512, 32] per step) so the recurrent matmul streams W_h as the moving
operand (2.4GHz) with h^T as cheap stationary tiles, and so h_seq^T tiles are
directly the stationary operand of the vocab projection.
"""

import numpy as np

B, T, K = 32, 64, 5
VOCAB, DE, DA, DI, H = 32000, 300, 300, 2048, 512
NCORE = 8
BL = B // NCORE          # batch rows per core
TOK = BL * T             # local tokens per core
TOKG = B * T             # global tokens
VS = VOCAB // NCORE      # vocab slice per core
H4 = 4 * H               # gate width
KX = DE + DI + DA        # non-recurrent contraction (2648)
NMT = TOKG // 128        # logits m-tiles (16)
STEPS_PER_MT = 128 // B  # 4 lstm steps per logits m-tile

_PROGRAM_CACHE = {}


# --------------------------------------------------------------------------
# Wait-split workaround: this walrus build accepts only one semaphore wait
# per CTRL instruction; hoist excess waits onto preceding no-ops.
# --------------------------------------------------------------------------
def _split_waits(nc, mybir, maxw=1):
    ctr = 0
    for f in nc.m.functions:
        for bb in f.blocks:
            new_insts = []
            for inst in bb.instructions:
                si = inst.sync_info
                if si is not None and si.on_wait and len(si.on_wait) > maxw:
                    waits = list(si.on_wait)
                    pre, keep = waits[:-maxw], waits[-maxw:]
                    for i in range(0, len(pre), maxw):
                        ctr += 1
                        nop = mybir.InstNoOp(
                            name=f"I-waitsplit-{ctr}",
                            engine=inst.engine,
                            ins=[],
                            outs=[],
                            sync_info=mybir.SyncInfo(
                                on_wait=list(pre[i:i + maxw]), on_update=[]),
                            text_hint="waitsplit",
                        )
                        new_insts.append(nop)
                        nc.register_instruction(nop, overwrite=True)
                    si.on_wait = keep
                new_insts.append(inst)
            bb.instructions = new_insts


# --------------------------------------------------------------------------
# Program builder
# --------------------------------------------------------------------------
def build_program(with_lstm_bias, with_logit_bias):
    import concourse.bass as bass
    import concourse.mybir as mybir
    import concourse.tile as tile
    from concourse.masks import make_identity

    f32 = mybir.dt.float32
    i32 = mybir.dt.int32
    AF = mybir.ActivationFunctionType
    OP = mybir.AluOpType

    nc = bass.Bass(num_devices=NCORE)

    # ---------------- I/O ----------------
    seq_tm = nc.declare_dram_parameter("seq_tm", [TOK], i32, isOutput=False)
    attr_ids = nc.declare_dram_parameter("attr_ids", [BL * K], i32, isOutput=False)
    img_T = nc.declare_dram_parameter("img_T", [DI, BL], f32, isOutput=False)
    lengths = nc.declare_dram_parameter("lengths", [B], i32, isOutput=False)
    word_emb = nc.declare_dram_parameter("word_emb", [VOCAB, DE], f32, isOutput=False)
    attr_emb = nc.declare_dram_parameter("attr_emb", [1000, DA], f32, isOutput=False)
    w_e = nc.declare_dram_parameter("w_e", [DE, H4], f32, isOutput=False)
    w_f = nc.declare_dram_parameter("w_f", [DI + DA, H4], f32, isOutput=False)
    w_h = nc.declare_dram_parameter("w_h", [H, H4], f32, isOutput=False)
    lstm_b = nc.declare_dram_parameter("lstm_b", [H4], f32, isOutput=False)
    w_log = nc.declare_dram_parameter("w_log", [H, VS], f32, isOutput=False)
    b_log = nc.declare_dram_parameter("b_log", [VS], f32, isOutput=False)

    out_logits = nc.declare_dram_parameter("out_logits", [TOKG, VS], f32, isOutput=True)
    out_max = nc.declare_dram_parameter("out_max", [128, NMT], f32, isOutput=True)
    out_arg = nc.declare_dram_parameter("out_arg", [128, NMT], i32, isOutput=True)

    # ---------------- constants (NEFF-embedded) ----------------
    # t index of token (p, c):  token = 128*c + p  ->  t = 4*c + p//32
    t_idx_np = np.empty((128, NMT), np.float32)
    for c in range(NMT):
        for p in range(128):
            t_idx_np[p, c] = 4 * c + p // 32
    t_idx_dram = nc.inline_tensor(t_idx_np, name="t_idx_const")
    # S^T[b, p] = 1 if p % BL == b : broadcasts per-batch G rows to token rows
    s_np = np.zeros((BL, 128), np.float32)
    for p in range(128):
        s_np[p % BL, p] = 1.0
    s_dram = nc.inline_tensor(s_np, name="s_const")
    # attr mean matrix: M[r, b] = 1/K if r // K == b  (r in [0, BL*K))
    m_np = np.zeros((BL * K, BL), np.float32)
    for r in range(BL * K):
        m_np[r, r // K] = 1.0 / K
    m_dram = nc.inline_tensor(m_np, name="m_const")
    ones_np = np.ones((1, 128), np.float32)
    ones_dram = nc.inline_tensor(ones_np, name="ones_const")

    NWE = 3                      # W_e k-tiles: 128,128,44
    WE_SZ = [128, 128, DE - 256]
    NWF = (DI + DA + 127) // 128  # 19 k-tiles of W_f (img 16 + attr 2.34)
    NKH = H // 128               # 4 k-tiles of h
    VCH = [512] * (VS // 512) + ([VS % 512] if VS % 512 else [])  # vocab chunks

    gate_funcs = [AF.Sigmoid, AF.Sigmoid, AF.Tanh, AF.Sigmoid]  # i, f, g, o
    chunk_order = [1, 0, 2, 3]  # process f, i, g, o

    with tile.TileContext(nc) as tc:
        dram_cm = tc.tile_pool(name="dram", bufs=1, space="DRAM")
        dram = dram_cm.__enter__()
        persist_cm = tc.tile_pool(name="persist", bufs=1)
        persist = persist_cm.__enter__()

        # ------------- persistent SBUF -------------
        identity = persist.tile([128, 128], f32)
        make_identity(nc, identity[:])
        w_h_sb = persist.tile([128, NKH, H4], f32)
        nc.sync.dma_start(w_h_sb[:], w_h.rearrange("(k p) n -> p k n", p=128))
        w_log_sb = persist.tile([128, NKH, VS], f32)
        nc.sync.dma_start(w_log_sb[:], w_log.rearrange("(k p) n -> p k n", p=128))
        h_seqT = persist.tile([128, NKH, TOKG], f32)
        lstmb_sb = None
        if with_lstm_bias:
            lstmb_sb = persist.tile([1, H4], f32)
            nc.sync.dma_start(lstmb_sb[:], lstm_b[None, :])
        blog_sb = None
        if with_logit_bias:
            blog_sb = persist.tile([1, VS], f32)
            nc.sync.dma_start(blog_sb[:], b_log[None, :])
        sT_sb = persist.tile([BL, 128], f32)
        nc.sync.dma_start(sT_sb[:], s_dram[:])
        mM_sb = persist.tile([BL * K, BL], f32)
        nc.sync.dma_start(mM_sb[:], m_dram[:])
        ones_sb = persist.tile([1, 128], f32)
        nc.sync.dma_start(ones_sb[:], ones_dram[:])
        mask_sb = persist.tile([128, NMT], f32)   # (t < len) per (p, mtile)
        amax_sb = persist.tile([128, NMT], f32)
        aarg_sb = persist.tile([128, NMT], f32)
        cst = [persist.tile([B, H], f32, name=f"c_state{i}") for i in range(2)]
        nc.vector.memset(cst[0][:], 0.0)

        # mask: t_idx < len  <=>  len > t_idx
        t_idx_sb = persist.tile([128, NMT], f32)
        nc.sync.dma_start(t_idx_sb[:], t_idx_dram[:])
        len_i = persist.tile([128, 1], i32)
        for r in range(128 // B):
            nc.sync.dma_start(len_i[r * B:(r + 1) * B, :], lengths[:, None])
        len_f = persist.tile([128, 1], f32)
        nc.vector.tensor_copy(len_f[:], len_i[:])
        nc.vector.tensor_tensor(
            out=mask_sb[:], in0=len_f[:].to_broadcast([128, NMT]),
            in1=t_idx_sb[:], op=OP.is_gt)

        bias_bc = None
        if with_logit_bias:
            bias_bc = persist.tile([128, VS], f32)

        # dram scratch for the AllGather (split: steps [0,8) first so the
        # recurrence can start while the big gather is still in flight)
        NT_A = 8
        ROWS_A = BL * NT_A
        zx_localA = dram.tile([ROWS_A, H4], f32)
        zx_localB = dram.tile([TOK - ROWS_A, H4], f32)
        zx_allA = dram.tile([NCORE, ROWS_A, H4], f32, addr_space="Shared")
        zx_allB = dram.tile([NCORE, TOK - ROWS_A, H4], f32, addr_space="Shared")

        # ================= Phase A: embeddings + Zx + AllGather =============
        with tc.tile_pool(name="phaseA", bufs=2) as pa, \
                tc.tile_pool(name="phaseA_w", bufs=2) as pa_w, \
                tc.tile_pool(name="phaseA_ps", bufs=2, space="PSUM") as pa_ps, \
                tc.tile_pool(name="phaseA_psg", bufs=1, space="PSUM") as pa_psg:
            ga_sb = pa.tile([BL, H4], f32, bufs=1)   # feats @ W_f (+bias)
            # ---- gathers ----
            attr_idx = pa.tile([BL * K, 1], i32, bufs=1)
            nc.sync.dma_start(attr_idx[:], attr_ids[:, None])
            attr_g = pa.tile([BL * K, DA], f32, bufs=1)
            nc.gpsimd.indirect_dma_start(
                out=attr_g[:], out_offset=None, in_=attr_emb[:, :],
                in_offset=bass.IndirectOffsetOnAxis(ap=attr_idx[:, :1], axis=0))

            widx = [pa.tile([128, 1], i32, name=f"widx{i}", bufs=1) for i in range(2)]
            wg = [pa.tile([128, DE], f32, name=f"wg{i}", bufs=1) for i in range(2)]
            for i in range(2):
                nc.sync.dma_start(widx[i][:], seq_tm[128 * i:128 * (i + 1), None])
                nc.gpsimd.indirect_dma_start(
                    out=wg[i][:], out_offset=None, in_=word_emb[:, :],
                    in_offset=bass.IndirectOffsetOnAxis(ap=widx[i][:, :1], axis=0))

            # ---- feats^T tiles [128, NWF, BL]: img_T then attr_mean^T ----
            featsT = pa.tile([128, NWF, BL], f32, bufs=1)
            nc.sync.dma_start(
                featsT[:, :DI // 128, :],
                img_T.rearrange("(k p) b -> p k b", p=128))
            # attr mean^T via PE: out = attr_g.T @ M  -> [DA, BL]
            for ft in range(3):
                fs = min(128, DA - 128 * ft)
                pm = pa_ps.tile([128, 128], f32, name="tr_ps", tag="tr_ps")
                nc.tensor.matmul(pm[:fs, :BL], attr_g[:, 128 * ft:128 * ft + fs],
                                 mM_sb[:], start=True, stop=True)
                nc.vector.tensor_copy(featsT[:fs, DI // 128 + ft, :], pm[:fs, :BL])

            # ---- x_word^T tiles [128, NWE, TOK] via PE transpose ----
            xwT = pa.tile([128, NWE, TOK], f32, bufs=1)
            for i in range(2):
                for ft in range(NWE):
                    fs = WE_SZ[ft]
                    pt = pa_ps.tile([128, 128], f32, name="tr_ps", tag="tr_ps")
                    nc.tensor.transpose(
                        out=pt[:fs, :], in_=wg[i][:, 128 * ft:128 * ft + fs],
                        identity=identity[:])
                    nc.vector.tensor_copy(xwT[:fs, ft, 128 * i:128 * (i + 1)],
                                          pt[:fs, :128])

            # ---- G = feats @ W_f (+ lstm bias), [BL, H4] ----
            gps = pa_psg.tile([BL, H4], f32)
            for kt in range(NWF):
                ks = min(128, DI + DA - 128 * kt)
                wft = pa_w.tile([128, H4], f32, name="wf_t")
                nc.sync.dma_start(wft[:ks, :], w_f[128 * kt:128 * kt + ks, :])
                for nch in range(4):
                    nc.tensor.matmul(
                        gps[:, 512 * nch:512 * (nch + 1)],
                        featsT[:ks, kt, :], wft[:ks, 512 * nch:512 * (nch + 1)],
                        start=(kt == 0), stop=(kt == NWF - 1 and not with_lstm_bias))
            if with_lstm_bias:
                for nch in range(4):
                    nc.tensor.matmul(
                        gps[:, 512 * nch:512 * (nch + 1)],
                        ones_sb[:, :BL], lstmb_sb[:, 512 * nch:512 * (nch + 1)],
                        start=False, stop=True)
            nc.vector.tensor_copy(ga_sb[:], gps[:])

            # ---- Zx local = x_word @ W_e + S@G, write to zx_local ----
            we_sb = pa.tile([128, NWE, H4], f32, bufs=1)
            for ft in range(NWE):
                fs = WE_SZ[ft]
                nc.sync.dma_start(we_sb[:fs, ft, :],
                                  w_e[128 * ft:128 * ft + fs, :])
            for i in range(2):
                for nch in range(4):
                    pz = pa_ps.tile([128, 512], f32, name="zx_ps")
                    for ft in range(NWE):
                        fs = WE_SZ[ft]
                        nc.tensor.matmul(
                            pz[:], xwT[:fs, ft, 128 * i:128 * (i + 1)],
                            we_sb[:fs, ft, 512 * nch:512 * (nch + 1)],
                            start=(ft == 0), stop=False)
                    nc.tensor.matmul(
                        pz[:], sT_sb[:], ga_sb[:, 512 * nch:512 * (nch + 1)],
                        start=False, stop=True)
                    zx_sb = pa.tile([128, 512], f32, name="zx_sb", bufs=3)
                    nc.vector.tensor_copy(zx_sb[:], pz[:])
                    sl_v = slice(512 * nch, 512 * (nch + 1))
                    if i == 0:
                        nc.sync.dma_start(zx_localA[:, sl_v], zx_sb[:ROWS_A, :])
                        nc.sync.dma_start(
                            zx_localB[:128 - ROWS_A, sl_v], zx_sb[ROWS_A:, :])
                    else:
                        nc.sync.dma_start(
                            zx_localB[128 - ROWS_A:256 - ROWS_A, sl_v], zx_sb[:])

            nc.gpsimd.collective_compute(
                "AllGather", OP.bypass,
                replica_groups=[list(range(NCORE))],
                ins=[zx_localA[:, :].opt()],
                outs=[zx_allA[:, :, :].opt()])
            nc.gpsimd.collective_compute(
                "AllGather", OP.bypass,
                replica_groups=[list(range(NCORE))],
                ins=[zx_localB[:, :].opt()],
                outs=[zx_allB[:, :, :].opt()])

        # ================= Phase B: recurrence + logits =====================
        with tc.tile_pool(name="phaseB", bufs=2) as pb, \
                tc.tile_pool(name="phaseB_zx", bufs=3) as pb_zx, \
                tc.tile_pool(name="phaseB_lsb", bufs=2) as pb_lsb, \
                tc.tile_pool(name="ps_z", bufs=2, space="PSUM") as ps_z, \
                tc.tile_pool(name="ps_tr", bufs=2, space="PSUM") as ps_tr, \
                tc.tile_pool(name="ps_l", bufs=4, space="PSUM") as ps_l:
            if with_logit_bias:
                for vc, vs in enumerate(VCH):
                    pbias = ps_l.tile([128, 512], f32, name="pl")
                    nc.tensor.matmul(pbias[:, :vs], ones_sb[:],
                                     blog_sb[:, 512 * vc:512 * vc + vs],
                                     start=True, stop=True)
                    nc.vector.tensor_copy(bias_bc[:, 512 * vc:512 * vc + vs],
                                          pbias[:, :vs])

            lsb_by_mt = {}
            argmax_state = {}

            def logits_mms(c_mt, vc):
                """PE part of one [128 tok, <=512 vocab] logits chunk."""
                vs = VCH[vc]
                msl = slice(128 * c_mt, 128 * (c_mt + 1))
                if c_mt not in lsb_by_mt:
                    lsb_by_mt[c_mt] = pb_lsb.tile([128, VS], f32, name="lsb")
                pl = ps_l.tile([128, 512], f32, name="pl")
                for kt in range(NKH):
                    nc.tensor.matmul(
                        pl[:, :vs], h_seqT[:, kt, msl],
                        w_log_sb[:, kt, 512 * vc:512 * vc + vs],
                        start=(kt == 0), stop=(kt == NKH - 1))
                return pl

            def logits_evict(c_mt, vc, pl):
                """Masked eviction (zero rows past the length) + store."""
                vs = VCH[vc]
                msl = slice(128 * c_mt, 128 * (c_mt + 1))
                lsb = lsb_by_mt[c_mt]
                nc.scalar.activation(
                    lsb[:, 512 * vc:512 * vc + vs], pl[:, :vs],
                    AF.Copy, scale=mask_sb[:, c_mt:c_mt + 1])
                if with_logit_bias:
                    nc.vector.tensor_tensor(
                        out=lsb[:, 512 * vc:512 * vc + vs],
                        in0=lsb[:, 512 * vc:512 * vc + vs],
                        in1=bias_bc[:, 512 * vc:512 * vc + vs],
                        op=OP.add)
                nc.sync.dma_start(
                    out_logits[msl, 512 * vc:512 * vc + vs],
                    lsb[:, 512 * vc:512 * vc + vs])

            HALF = VS // 2

            def emit_argmax_part(c_mt, s):
                """Argmax over vocab-halves, spread over 4 steps' DVE slack."""
                lsb = lsb_by_mt[c_mt]
                if s == 0:
                    st = argmax_state[c_mt] = {
                        "mx": pb.tile([128, 2, 8], f32, name="mx8", bufs=2),
                        "ix": pb.tile([128, 2, 8], mybir.dt.uint32, name="ix8",
                                      bufs=2),
                    }
                    nc.vector.max(out=st["mx"][:, 0, :], in_=lsb[:, :HALF])
                elif s == 1:
                    st = argmax_state[c_mt]
                    nc.vector.max(out=st["mx"][:, 1, :], in_=lsb[:, HALF:])
                elif s == 2:
                    st = argmax_state[c_mt]
                    nc.vector.max_index(out=st["ix"][:, 0, :],
                                        in_max=st["mx"][:, 0, :],
                                        in_values=lsb[:, :HALF])
                else:
                    st = argmax_state.pop(c_mt)
                    nc.vector.max_index(out=st["ix"][:, 1, :],
                                        in_max=st["mx"][:, 1, :],
                                        in_values=lsb[:, HALF:])
                    lsb_by_mt.pop(c_mt)
                    # merge halves: prefer the low half on ties (jnp.argmax)
                    vlo, vhi = st["mx"][:, 0, :1], st["mx"][:, 1, :1]
                    sel = pb.tile([128, 1], f32, name="sel", bufs=2)
                    nc.vector.tensor_tensor(out=sel[:], in0=vhi, in1=vlo,
                                            op=OP.is_gt)
                    nc.vector.tensor_tensor(out=amax_sb[:, c_mt:c_mt + 1],
                                            in0=vlo, in1=vhi, op=OP.max)
                    ilo = pb.tile([128, 1], f32, name="ilo", bufs=2)
                    nc.vector.tensor_copy(ilo[:], st["ix"][:, 0, :1])
                    ihi = pb.tile([128, 1], f32, name="ihi", bufs=2)
                    nc.vector.tensor_copy(ihi[:], st["ix"][:, 1, :1])
                    nc.vector.tensor_scalar(ihi[:], ihi[:], float(HALF), None,
                                            op0=OP.add)
                    ixf = pb.tile([128, 1], f32, name="ixf", bufs=2)
                    # ixf = sel ? ihi : ilo  =  ilo + sel*(ihi-ilo)
                    nc.vector.tensor_tensor(out=ixf[:], in0=ihi, in1=ilo,
                                            op=OP.subtract)
                    nc.vector.tensor_tensor(out=ixf[:], in0=ixf[:], in1=sel[:],
                                            op=OP.mult)
                    nc.vector.tensor_tensor(out=ixf[:], in0=ixf[:], in1=ilo[:],
                                            op=OP.add)
                    if not with_logit_bias:
                        # all-masked rows are all-zero; force argmax to 0 to
                        # match jnp.argmax's lowest-index tie-break
                        nc.vector.tensor_tensor(
                            out=ixf[:], in0=ixf[:],
                            in1=mask_sb[:, c_mt:c_mt + 1], op=OP.mult)
                    nc.vector.tensor_copy(aarg_sb[:, c_mt:c_mt + 1], ixf[:])

            HCH = 128                      # h is produced in 128-col chunks
            for t in range(T):
                c_prev, c_new = cst[t % 2], cst[(t + 1) % 2]
                s = t % STEPS_PER_MT
                c_log = t // STEPS_PER_MT - 1      # m-tile getting its logits
                c_arg = t // STEPS_PER_MT - 2      # m-tile getting its argmax

                zx_t = pb_zx.tile([B, H4], f32, name="zx_t")
                if t < NT_A:
                    nc.sync.dma_start(zx_t[:], zx_allA[:, BL * t:BL * (t + 1), :])
                else:
                    tb = t - NT_A
                    nc.sync.dma_start(zx_t[:], zx_allB[:, BL * tb:BL * (tb + 1), :])

                # --- recurrent matmuls: the four 512-wide gate chunks run
                # CONCURRENTLY in the four 32-column groups of the PE array
                # (M=32 per chunk; tile_position col-tiling) ---
                pz4 = ps_z.tile([128, 512], f32, name="z_ps")
                for ci in range(4):
                    # each column-group starts its own accumulation group on
                    # its own 32 partitions; the sim's group checker doesn't
                    # understand partition-offset groups, so skip it.
                    nc.tensor.matmul(pz4[32 * ci:32 * (ci + 1), :],
                                     identity[:B, :B],
                                     zx_t[:, 512 * ci:512 * (ci + 1)],
                                     start=True, stop=(t == 0),
                                     tile_position=(0, 32 * ci),
                                     skip_group_check=True)
                if t > 0:
                    hT_prev = h_seqT[:, :, B * (t - 1):B * t]
                    for kt in range(NKH):
                        for ci in range(4):
                            nc.tensor.matmul(
                                pz4[32 * ci:32 * (ci + 1), :],
                                hT_prev[:, kt, :],
                                w_h_sb[:, kt, 512 * ci:512 * (ci + 1)],
                                start=False, stop=(kt == NKH - 1),
                                tile_position=(0, 32 * ci),
                                skip_group_check=True)
                gates = {}
                for ci in chunk_order:
                    g_sb = pb.tile([B, 512], f32, name=f"gate{ci}", bufs=1)
                    nc.scalar.activation(g_sb[:], pz4[32 * ci:32 * (ci + 1), :],
                                         gate_funcs[ci])
                    gates[ci] = g_sb

                # --- logits matmuls fill the PE while ACT/DVE run the chain ---
                pls = []
                if c_log >= 0:
                    pls.append((2 * s, logits_mms(c_log, 2 * s)))
                    pls.append((2 * s + 1, logits_mms(c_log, 2 * s + 1)))
                # argmax halves of an older m-tile run in DVE slack; emit
                # before the chain so they don't block it in the DVE FIFO
                if c_arg >= 0:
                    emit_argmax_part(c_arg, s)

                # --- c' = sig(f)*c + sig(i)*tanh(g);  h = sig(o)*tanh(c') ---
                t_a = pb.tile([B, H], f32, name="t_a", bufs=1)
                nc.vector.tensor_tensor(out=t_a[:], in0=gates[1][:],
                                        in1=c_prev[:], op=OP.mult)
                t_b = pb.tile([B, H], f32, name="t_b", bufs=1)
                nc.gpsimd.tensor_tensor(out=t_b[:], in0=gates[0][:],
                                        in1=gates[2][:], op=OP.mult)
                nc.vector.tensor_tensor(out=c_new[:], in0=t_a[:], in1=t_b[:],
                                        op=OP.add)
                # tanh/h/transpose pipelined in 128-wide chunks so the next
                # step's first matmuls can begin before the whole h is done
                t_th = pb.tile([B, H], f32, name="t_th", bufs=1)
                h_sb = pb.tile([B, H], f32, name="h_sb", bufs=2)
                for kt in range(NKH):
                    hsl = slice(HCH * kt, HCH * (kt + 1))
                    nc.scalar.activation(t_th[:, hsl], c_new[:, hsl], AF.Tanh)
                    nc.vector.tensor_tensor(out=h_sb[:, hsl],
                                            in0=gates[3][:, hsl],
                                            in1=t_th[:, hsl], op=OP.mult)
                    ptr = ps_tr.tile([128, B], f32, name="tr_ps")
                    nc.tensor.transpose(out=ptr[:], in_=h_sb[:, hsl],
                                        identity=identity[:B, :B])
                    if kt % 2 == 0:
                        nc.vector.tensor_copy(h_seqT[:, kt, B * t:B * (t + 1)],
                                              ptr[:])
                    else:
                        nc.scalar.copy(h_seqT[:, kt, B * t:B * (t + 1)], ptr[:])

                # --- evictions late (ACT slack), argmax parts (DVE slack) ---
                for vc, pl in pls:
                    logits_evict(c_log, vc, pl)

            # tail: last m-tile + remaining argmax parts
            for vc in range(len(VCH)):
                logits_evict(NMT - 1, vc, logits_mms(NMT - 1, vc))
            for s in range(4):
                emit_argmax_part(NMT - 2, s)
            for s in range(4):
                emit_argmax_part(NMT - 1, s)

            aarg_i = persist.tile([128, NMT], i32)
            nc.vector.tensor_copy(aarg_i[:], aarg_sb[:])
            nc.sync.dma_start(out_max[:, :], amax_sb[:])
            nc.sync.dma_start(out_arg[:, :], aarg_i[:])

        persist_cm.__exit__(None, None, None)
        dram_cm.__exit__(None, None, None)

    _split_waits(nc, mybir)
    return nc


def _get_program(with_lstm_bias, with_logit_bias):
    key = (with_lstm_bias, with_logit_bias)
    if key not in _PROGRAM_CACHE:
        _PROGRAM_CACHE[key] = build_program(*key)
    return _PROGRAM_CACHE[key]


# --------------------------------------------------------------------------
# Host wrapper
# --------------------------------------------------------------------------
def make_in_maps(top_k_attributes, mean_image_features, seq_inputs, lengths,
                 word_emb, attr_emb, lstm_kernel, lstm_bias, W_logits, b_logits):
    asnp = lambda x: np.ascontiguousarray(np.asarray(x))
    top_k_attributes = asnp(top_k_attributes).astype(np.int32)
    seq_inputs = asnp(seq_inputs).astype(np.int32)
    lengths_np = asnp(lengths).astype(np.int32)
    mean_image_features = asnp(mean_image_features).astype(np.float32)
    word_emb = asnp(word_emb).astype(np.float32)
    attr_emb = asnp(attr_emb).astype(np.float32)
    lstm_kernel = asnp(lstm_kernel).astype(np.float32)
    lstm_bias = asnp(lstm_bias).astype(np.float32)
    W_logits = asnp(W_logits).astype(np.float32)
    b_logits = asnp(b_logits).astype(np.float32)

    w_e = np.ascontiguousarray(lstm_kernel[:DE])
    w_f = np.ascontiguousarray(lstm_kernel[DE:DE + DI + DA])
    w_h = np.ascontiguousarray(lstm_kernel[DE + DI + DA:])

    in_maps = []
    for j in range(NCORE):
        bs = slice(BL * j, BL * (j + 1))
        vs = slice(VS * j, VS * (j + 1))
        seq_j = seq_inputs[bs]                       # [BL, T]
        in_maps.append({
            "seq_tm": np.ascontiguousarray(seq_j.T.reshape(-1)),   # t-major
            "attr_ids": np.ascontiguousarray(top_k_attributes[bs].reshape(-1)),
            "img_T": np.ascontiguousarray(mean_image_features[bs].T),
            "lengths": lengths_np,
            "word_emb": word_emb,
            "attr_emb": attr_emb,
            "w_e": w_e,
            "w_f": w_f,
            "w_h": w_h,
            "lstm_b": lstm_bias,
            "w_log": np.ascontiguousarray(W_logits[:, vs]),
            "b_log": np.ascontiguousarray(b_logits[vs]),
        })
    flags = (bool(np.any(lstm_bias)), bool(np.any(b_logits)))
    return in_maps, flags


def combine_outputs(results):
    logits = np.empty((B, T, VOCAB), np.float32)
    vals = np.empty((NCORE, TOKG), np.float32)
    args = np.empty((NCORE, TOKG), np.int64)
    for j, r in enumerate(results):
        lj = r["out_logits"].reshape(T, B, VS)        # rows are t*32+b
        logits[:, :, VS * j:VS * (j + 1)] = lj.transpose(1, 0, 2)
        vals[j] = r["out_max"].T.reshape(-1)          # token = 128*c + p
        args[j] = r["out_arg"].T.reshape(-1)
    win = np.argmax(vals, axis=0)                     # ties -> lowest core
    tok = np.arange(TOKG)
    gl_arg = (args[win, tok] + VS * win).astype(np.int32)
    preds = gl_arg.reshape(T, B).T.copy()             # [B, T]
    return logits, preds


def kernel(**inputs):
    from concourse.bass_utils import run_bass_kernel_spmd
    in_maps, flags = make_in_maps(**inputs)
    nc = _get_program(*flags)
    res = run_bass_kernel_spmd(nc, in_maps, list(range(NCORE)))
    return combine_outputs(res.results)


# expose for test.py profiling
def kernel_traced(**inputs):
    from concourse.bass_utils import run_bass_kernel_spmd
    in_maps, flags = make_in_maps(**inputs)
    nc = _get_program(*flags)
    res = run_bass_kernel_spmd(nc, in_maps, list(range(NCORE)), trace=True)
    return combine_outputs(res.results), res


# revision 22
# speedup vs baseline: 1.0870x; 1.0870x over previous
"""AttributeImageCaptioner fused kernel for 8 trn2 NeuronCores.

Model (see reference):
  attr/word embedding gathers -> per-step LSTM (T=64, B=32, H=512) over
  inp = [word_emb(300) | image_feats(2048) | attr_mean(300)] -> masked h_seq
  -> vocab projection [512, 32000] (+argmax).

Distribution strategy:
  * The non-recurrent 84% of the LSTM contraction (inp @ W[:2648]) is
    batch-sharded: each core computes Zx for its 4 batch rows, then an
    AllGather shares the full Zx [2048 tok, 2048 gates].
  * The recurrence (h @ W[2648:3160] + gates) is replicated on all cores
    (per-step collectives are slower than the 3.4us/step of replicated work).
  * The vocab projection + argmax is vocab-sharded: each core holds a
    [512, 4000] slice of W_logits in SBUF and writes 1/8 of the logits.
  * Host combines per-core (max, argmax) candidates and concatenates logits.

Layout notes: everything downstream of Zx lives in "transposed" form
(h^T [512, 32] per step) so the recurrent matmul streams W_h as the moving
operand (2.4GHz) with h^T as cheap stationary tiles, and so h_seq^T tiles are
directly the stationary operand of the vocab projection.
"""

import numpy as np

B, T, K = 32, 64, 5
VOCAB, DE, DA, DI, H = 32000, 300, 300, 2048, 512
NCORE = 8
BL = B // NCORE          # batch rows per core
TOK = BL * T             # local tokens per core
TOKG = B * T             # global tokens
VS = VOCAB // NCORE      # vocab slice per core
H4 = 4 * H               # gate width
KX = DE + DI + DA        # non-recurrent contraction (2648)
NMT = TOKG // 128        # logits m-tiles (16)
STEPS_PER_MT = 128 // B  # 4 lstm steps per logits m-tile

_PROGRAM_CACHE = {}


# --------------------------------------------------------------------------
# Wait-split workaround: this walrus build accepts only one semaphore wait
# per CTRL instruction; hoist excess waits onto preceding no-ops.
# --------------------------------------------------------------------------
def _split_waits(nc, mybir, maxw=1):
    ctr = 0
    for f in nc.m.functions:
        for bb in f.blocks:
            new_insts = []
            for inst in bb.instructions:
                si = inst.sync_info
                if si is not None and si.on_wait and len(si.on_wait) > maxw:
                    waits = list(si.on_wait)
                    pre, keep = waits[:-maxw], waits[-maxw:]
                    for i in range(0, len(pre), maxw):
                        ctr += 1
                        nop = mybir.InstNoOp(
                            name=f"I-waitsplit-{ctr}",
                            engine=inst.engine,
                            ins=[],
                            outs=[],
                            sync_info=mybir.SyncInfo(
                                on_wait=list(pre[i:i + maxw]), on_update=[]),
                            text_hint="waitsplit",
                        )
                        new_insts.append(nop)
                        nc.register_instruction(nop, overwrite=True)
                    si.on_wait = keep
                new_insts.append(inst)
            bb.instructions = new_insts


# --------------------------------------------------------------------------
# Program builder
# --------------------------------------------------------------------------
def build_program(with_lstm_bias, with_logit_bias):
    import concourse.bass as bass
    import concourse.mybir as mybir
    import concourse.tile as tile
    from concourse.masks import make_identity

    f32 = mybir.dt.float32
    i32 = mybir.dt.int32
    AF = mybir.ActivationFunctionType
    OP = mybir.AluOpType

    nc = bass.Bass(num_devices=NCORE)

    # ---------------- I/O ----------------
    seq_tm = nc.declare_dram_parameter("seq_tm", [TOK], i32, isOutput=False)
    attr_ids = nc.declare_dram_parameter("attr_ids", [BL * K], i32, isOutput=False)
    img_T = nc.declare_dram_parameter("img_T", [DI, BL], f32, isOutput=False)
    lengths = nc.declare_dram_parameter("lengths", [B], i32, isOutput=False)
    word_emb = nc.declare_dram_parameter("word_emb", [VOCAB, DE], f32, isOutput=False)
    attr_emb = nc.declare_dram_parameter("attr_emb", [1000, DA], f32, isOutput=False)
    w_e = nc.declare_dram_parameter("w_e", [DE, H4], f32, isOutput=False)
    w_f = nc.declare_dram_parameter("w_f", [DI + DA, H4], f32, isOutput=False)
    w_h = nc.declare_dram_parameter("w_h", [H, H4], f32, isOutput=False)
    lstm_b = nc.declare_dram_parameter("lstm_b", [H4], f32, isOutput=False)
    w_log = nc.declare_dram_parameter("w_log", [H, VS], f32, isOutput=False)
    b_log = nc.declare_dram_parameter("b_log", [VS], f32, isOutput=False)

    out_logits = nc.declare_dram_parameter("out_logits", [TOKG, VS], f32, isOutput=True)
    out_max = nc.declare_dram_parameter("out_max", [128, NMT], f32, isOutput=True)
    out_arg = nc.declare_dram_parameter("out_arg", [128, NMT], i32, isOutput=True)

    # ---------------- constants (NEFF-embedded) ----------------
    # t index of token (p, c):  token = 128*c + p  ->  t = 4*c + p//32
    t_idx_np = np.empty((128, NMT), np.float32)
    for c in range(NMT):
        for p in range(128):
            t_idx_np[p, c] = 4 * c + p // 32
    t_idx_dram = nc.inline_tensor(t_idx_np, name="t_idx_const")
    # S^T[b, p] = 1 if p % BL == b : broadcasts per-batch G rows to token rows
    s_np = np.zeros((BL, 128), np.float32)
    for p in range(128):
        s_np[p % BL, p] = 1.0
    s_dram = nc.inline_tensor(s_np, name="s_const")
    # attr mean matrix: M[r, b] = 1/K if r // K == b  (r in [0, BL*K))
    m_np = np.zeros((BL * K, BL), np.float32)
    for r in range(BL * K):
        m_np[r, r // K] = 1.0 / K
    m_dram = nc.inline_tensor(m_np, name="m_const")
    ones_np = np.ones((1, 128), np.float32)
    ones_dram = nc.inline_tensor(ones_np, name="ones_const")

    NWE = 3                      # W_e k-tiles: 128,128,44
    WE_SZ = [128, 128, DE - 256]
    NWF = (DI + DA + 127) // 128  # 19 k-tiles of W_f (img 16 + attr 2.34)
    NKH = H // 128               # 4 k-tiles of h
    VCH = [512] * (VS // 512) + ([VS % 512] if VS % 512 else [])  # vocab chunks

    gate_funcs = [AF.Sigmoid, AF.Sigmoid, AF.Tanh, AF.Sigmoid]  # i, f, g, o
    chunk_order = [1, 0, 2, 3]  # process f, i, g, o

    with tile.TileContext(nc) as tc:
        dram_cm = tc.tile_pool(name="dram", bufs=1, space="DRAM")
        dram = dram_cm.__enter__()
        persist_cm = tc.tile_pool(name="persist", bufs=1)
        persist = persist_cm.__enter__()

        # ------------- persistent SBUF -------------
        identity = persist.tile([128, 128], f32)
        make_identity(nc, identity[:])
        w_h_sb = persist.tile([128, NKH, H4], f32)
        nc.sync.dma_start(w_h_sb[:], w_h.rearrange("(k p) n -> p k n", p=128))
        w_log_sb = persist.tile([128, NKH, VS], f32)
        nc.sync.dma_start(w_log_sb[:], w_log.rearrange("(k p) n -> p k n", p=128))
        h_seqT = persist.tile([128, NKH, TOKG], f32)
        lstmb_sb = None
        if with_lstm_bias:
            lstmb_sb = persist.tile([1, H4], f32)
            nc.sync.dma_start(lstmb_sb[:], lstm_b[None, :])
        blog_sb = None
        if with_logit_bias:
            blog_sb = persist.tile([1, VS], f32)
            nc.sync.dma_start(blog_sb[:], b_log[None, :])
        sT_sb = persist.tile([BL, 128], f32)
        nc.sync.dma_start(sT_sb[:], s_dram[:])
        mM_sb = persist.tile([BL * K, BL], f32)
        nc.sync.dma_start(mM_sb[:], m_dram[:])
        ones_sb = persist.tile([1, 128], f32)
        nc.sync.dma_start(ones_sb[:], ones_dram[:])
        mask_sb = persist.tile([128, NMT], f32)   # (t < len) per (p, mtile)
        amax_sb = persist.tile([128, NMT], f32)
        aarg_sb = persist.tile([128, NMT], f32)
        cst = [persist.tile([B, H], f32, name=f"c_state{i}") for i in range(2)]
        nc.vector.memset(cst[0][:], 0.0)

        # mask: t_idx < len  <=>  len > t_idx
        t_idx_sb = persist.tile([128, NMT], f32)
        nc.sync.dma_start(t_idx_sb[:], t_idx_dram[:])
        len_i = persist.tile([128, 1], i32)
        for r in range(128 // B):
            nc.sync.dma_start(len_i[r * B:(r + 1) * B, :], lengths[:, None])
        len_f = persist.tile([128, 1], f32)
        nc.vector.tensor_copy(len_f[:], len_i[:])
        nc.vector.tensor_tensor(
            out=mask_sb[:], in0=len_f[:].to_broadcast([128, NMT]),
            in1=t_idx_sb[:], op=OP.is_gt)

        bias_bc = None
        if with_logit_bias:
            bias_bc = persist.tile([128, VS], f32)

        # dram scratch for the AllGather (split: steps [0,8) first so the
        # recurrence can start while the big gather is still in flight)
        NT_A = 8
        ROWS_A = BL * NT_A
        zx_localA = dram.tile([ROWS_A, H4], f32)
        zx_localB = dram.tile([TOK - ROWS_A, H4], f32)
        zx_allA = dram.tile([NCORE, ROWS_A, H4], f32, addr_space="Shared")
        zx_allB = dram.tile([NCORE, TOK - ROWS_A, H4], f32, addr_space="Shared")

        # ================= Phase A: embeddings + Zx + AllGather =============
        with tc.tile_pool(name="phaseA", bufs=2) as pa, \
                tc.tile_pool(name="phaseA_w", bufs=2) as pa_w, \
                tc.tile_pool(name="phaseA_ps", bufs=2, space="PSUM") as pa_ps, \
                tc.tile_pool(name="phaseA_psg", bufs=1, space="PSUM") as pa_psg:
            ga_sb = pa.tile([BL, H4], f32, bufs=1)   # feats @ W_f (+bias)
            # ---- gathers ----
            attr_idx = pa.tile([BL * K, 1], i32, bufs=1)
            nc.sync.dma_start(attr_idx[:], attr_ids[:, None])
            attr_g = pa.tile([BL * K, DA], f32, bufs=1)
            nc.gpsimd.indirect_dma_start(
                out=attr_g[:], out_offset=None, in_=attr_emb[:, :],
                in_offset=bass.IndirectOffsetOnAxis(ap=attr_idx[:, :1], axis=0))

            widx = [pa.tile([128, 1], i32, name=f"widx{i}", bufs=1) for i in range(2)]
            wg = [pa.tile([128, DE], f32, name=f"wg{i}", bufs=1) for i in range(2)]
            for i in range(2):
                nc.sync.dma_start(widx[i][:], seq_tm[128 * i:128 * (i + 1), None])
                nc.gpsimd.indirect_dma_start(
                    out=wg[i][:], out_offset=None, in_=word_emb[:, :],
                    in_offset=bass.IndirectOffsetOnAxis(ap=widx[i][:, :1], axis=0))

            # ---- feats^T tiles [128, NWF, BL]: img_T then attr_mean^T ----
            featsT = pa.tile([128, NWF, BL], f32, bufs=1)
            nc.sync.dma_start(
                featsT[:, :DI // 128, :],
                img_T.rearrange("(k p) b -> p k b", p=128))
            # attr mean^T via PE: out = attr_g.T @ M  -> [DA, BL]
            for ft in range(3):
                fs = min(128, DA - 128 * ft)
                pm = pa_ps.tile([128, 128], f32, name="tr_ps", tag="tr_ps")
                nc.tensor.matmul(pm[:fs, :BL], attr_g[:, 128 * ft:128 * ft + fs],
                                 mM_sb[:], start=True, stop=True)
                nc.vector.tensor_copy(featsT[:fs, DI // 128 + ft, :], pm[:fs, :BL])

            # ---- x_word^T tiles [128, NWE, TOK] via PE transpose ----
            xwT = pa.tile([128, NWE, TOK], f32, bufs=1)
            for i in range(2):
                for ft in range(NWE):
                    fs = WE_SZ[ft]
                    pt = pa_ps.tile([128, 128], f32, name="tr_ps", tag="tr_ps")
                    nc.tensor.transpose(
                        out=pt[:fs, :], in_=wg[i][:, 128 * ft:128 * ft + fs],
                        identity=identity[:])
                    nc.vector.tensor_copy(xwT[:fs, ft, 128 * i:128 * (i + 1)],
                                          pt[:fs, :128])

            # ---- G = feats @ W_f (+ lstm bias), [BL, H4] ----
            gps = pa_psg.tile([BL, H4], f32)
            for kt in range(NWF):
                ks = min(128, DI + DA - 128 * kt)
                wft = pa_w.tile([128, H4], f32, name="wf_t")
                nc.sync.dma_start(wft[:ks, :], w_f[128 * kt:128 * kt + ks, :])
                for nch in range(4):
                    nc.tensor.matmul(
                        gps[:, 512 * nch:512 * (nch + 1)],
                        featsT[:ks, kt, :], wft[:ks, 512 * nch:512 * (nch + 1)],
                        start=(kt == 0), stop=(kt == NWF - 1 and not with_lstm_bias))
            if with_lstm_bias:
                for nch in range(4):
                    nc.tensor.matmul(
                        gps[:, 512 * nch:512 * (nch + 1)],
                        ones_sb[:, :BL], lstmb_sb[:, 512 * nch:512 * (nch + 1)],
                        start=False, stop=True)
            nc.vector.tensor_copy(ga_sb[:], gps[:])

            # ---- Zx local = x_word @ W_e + S@G, write to zx_local ----
            we_sb = pa.tile([128, NWE, H4], f32, bufs=1)
            for ft in range(NWE):
                fs = WE_SZ[ft]
                nc.sync.dma_start(we_sb[:fs, ft, :],
                                  w_e[128 * ft:128 * ft + fs, :])
            for i in range(2):
                for nch in range(4):
                    pz = pa_ps.tile([128, 512], f32, name="zx_ps")
                    for ft in range(NWE):
                        fs = WE_SZ[ft]
                        nc.tensor.matmul(
                            pz[:], xwT[:fs, ft, 128 * i:128 * (i + 1)],
                            we_sb[:fs, ft, 512 * nch:512 * (nch + 1)],
                            start=(ft == 0), stop=False)
                    nc.tensor.matmul(
                        pz[:], sT_sb[:], ga_sb[:, 512 * nch:512 * (nch + 1)],
                        start=False, stop=True)
                    zx_sb = pa.tile([128, 512], f32, name="zx_sb", bufs=3)
                    nc.vector.tensor_copy(zx_sb[:], pz[:])
                    sl_v = slice(512 * nch, 512 * (nch + 1))
                    if i == 0:
                        nc.sync.dma_start(zx_localA[:, sl_v], zx_sb[:ROWS_A, :])
                        nc.sync.dma_start(
                            zx_localB[:128 - ROWS_A, sl_v], zx_sb[ROWS_A:, :])
                    else:
                        nc.sync.dma_start(
                            zx_localB[128 - ROWS_A:256 - ROWS_A, sl_v], zx_sb[:])

            nc.gpsimd.collective_compute(
                "AllGather", OP.bypass,
                replica_groups=[list(range(NCORE))],
                ins=[zx_localA[:, :].opt()],
                outs=[zx_allA[:, :, :].opt()])
            nc.gpsimd.collective_compute(
                "AllGather", OP.bypass,
                replica_groups=[list(range(NCORE))],
                ins=[zx_localB[:, :].opt()],
                outs=[zx_allB[:, :, :].opt()])

        # ================= Phase B: recurrence + logits =====================
        with tc.tile_pool(name="phaseB", bufs=2) as pb, \
                tc.tile_pool(name="phaseB_zx", bufs=3) as pb_zx, \
                tc.tile_pool(name="phaseB_lsb", bufs=2) as pb_lsb, \
                tc.tile_pool(name="ps_z", bufs=2, space="PSUM") as ps_z, \
                tc.tile_pool(name="ps_tr", bufs=2, space="PSUM") as ps_tr, \
                tc.tile_pool(name="ps_l", bufs=4, space="PSUM") as ps_l:
            if with_logit_bias:
                for vc, vs in enumerate(VCH):
                    pbias = ps_l.tile([128, 512], f32, name="pl")
                    nc.tensor.matmul(pbias[:, :vs], ones_sb[:],
                                     blog_sb[:, 512 * vc:512 * vc + vs],
                                     start=True, stop=True)
                    nc.vector.tensor_copy(bias_bc[:, 512 * vc:512 * vc + vs],
                                          pbias[:, :vs])

            lsb_by_mt = {}
            argmax_state = {}

            def logits_mms(c_mt, vc):
                """PE part of one [128 tok, <=512 vocab] logits chunk."""
                vs = VCH[vc]
                msl = slice(128 * c_mt, 128 * (c_mt + 1))
                if c_mt not in lsb_by_mt:
                    lsb_by_mt[c_mt] = pb_lsb.tile([128, VS], f32, name="lsb")
                pl = ps_l.tile([128, 512], f32, name="pl")
                for kt in range(NKH):
                    nc.tensor.matmul(
                        pl[:, :vs], h_seqT[:, kt, msl],
                        w_log_sb[:, kt, 512 * vc:512 * vc + vs],
                        start=(kt == 0), stop=(kt == NKH - 1))
                return pl

            def logits_evict(c_mt, vc, pl):
                """Masked eviction (zero rows past the length) + store."""
                vs = VCH[vc]
                msl = slice(128 * c_mt, 128 * (c_mt + 1))
                lsb = lsb_by_mt[c_mt]
                nc.scalar.activation(
                    lsb[:, 512 * vc:512 * vc + vs], pl[:, :vs],
                    AF.Copy, scale=mask_sb[:, c_mt:c_mt + 1])
                if with_logit_bias:
                    nc.vector.tensor_tensor(
                        out=lsb[:, 512 * vc:512 * vc + vs],
                        in0=lsb[:, 512 * vc:512 * vc + vs],
                        in1=bias_bc[:, 512 * vc:512 * vc + vs],
                        op=OP.add)
                nc.sync.dma_start(
                    out_logits[msl, 512 * vc:512 * vc + vs],
                    lsb[:, 512 * vc:512 * vc + vs])

            HALF = VS // 2

            def emit_argmax_part(c_mt, s):
                """Argmax over vocab-halves, spread over 4 steps' DVE slack."""
                lsb = lsb_by_mt[c_mt]
                if s == 0:
                    st = argmax_state[c_mt] = {
                        "mx": pb.tile([128, 2, 8], f32, name="mx8", bufs=2),
                        "ix": pb.tile([128, 2, 8], mybir.dt.uint32, name="ix8",
                                      bufs=2),
                    }
                    nc.vector.max(out=st["mx"][:, 0, :], in_=lsb[:, :HALF])
                elif s == 1:
                    st = argmax_state[c_mt]
                    nc.vector.max(out=st["mx"][:, 1, :], in_=lsb[:, HALF:])
                elif s == 2:
                    st = argmax_state[c_mt]
                    nc.vector.max_index(out=st["ix"][:, 0, :],
                                        in_max=st["mx"][:, 0, :],
                                        in_values=lsb[:, :HALF])
                else:
                    st = argmax_state.pop(c_mt)
                    nc.vector.max_index(out=st["ix"][:, 1, :],
                                        in_max=st["mx"][:, 1, :],
                                        in_values=lsb[:, HALF:])
                    lsb_by_mt.pop(c_mt)
                    # merge halves: prefer the low half on ties (jnp.argmax)
                    vlo, vhi = st["mx"][:, 0, :1], st["mx"][:, 1, :1]
                    sel = pb.tile([128, 1], f32, name="sel", bufs=2)
                    nc.vector.tensor_tensor(out=sel[:], in0=vhi, in1=vlo,
                                            op=OP.is_gt)
                    nc.vector.tensor_tensor(out=amax_sb[:, c_mt:c_mt + 1],
                                            in0=vlo, in1=vhi, op=OP.max)
                    ilo = pb.tile([128, 1], f32, name="ilo", bufs=2)
                    nc.vector.tensor_copy(ilo[:], st["ix"][:, 0, :1])
                    ihi = pb.tile([128, 1], f32, name="ihi", bufs=2)
                    nc.vector.tensor_copy(ihi[:], st["ix"][:, 1, :1])
                    nc.vector.tensor_scalar(ihi[:], ihi[:], float(HALF), None,
                                            op0=OP.add)
                    ixf = pb.tile([128, 1], f32, name="ixf", bufs=2)
                    # ixf = sel ? ihi : ilo  =  ilo + sel*(ihi-ilo)
                    nc.vector.tensor_tensor(out=ixf[:], in0=ihi, in1=ilo,
                                            op=OP.subtract)
                    nc.vector.tensor_tensor(out=ixf[:], in0=ixf[:], in1=sel[:],
                                            op=OP.mult)
                    nc.vector.tensor_tensor(out=ixf[:], in0=ixf[:], in1=ilo[:],
                                            op=OP.add)
                    if not with_logit_bias:
                        # all-masked rows are all-zero; force argmax to 0 to
                        # match jnp.argmax's lowest-index tie-break
                        nc.vector.tensor_tensor(
                            out=ixf[:], in0=ixf[:],
                            in1=mask_sb[:, c_mt:c_mt + 1], op=OP.mult)
                    nc.vector.tensor_copy(aarg_sb[:, c_mt:c_mt + 1], ixf[:])

            HCH = 128                      # h is produced in 128-col chunks
            for t in range(T):
                c_prev, c_new = cst[t % 2], cst[(t + 1) % 2]
                s = t % STEPS_PER_MT
                c_log = t // STEPS_PER_MT - 1      # m-tile getting its logits
                c_arg = t // STEPS_PER_MT - 2      # m-tile getting its argmax

                zx_t = pb_zx.tile([B, H4], f32, name="zx_t")
                if t < NT_A:
                    nc.sync.dma_start(zx_t[:], zx_allA[:, BL * t:BL * (t + 1), :])
                else:
                    tb = t - NT_A
                    nc.sync.dma_start(zx_t[:], zx_allB[:, BL * tb:BL * (tb + 1), :])

                # --- recurrent matmuls: the four 512-wide gate chunks run
                # CONCURRENTLY in the four 32-column groups of the PE array
                # (M=32 per chunk; tile_position col-tiling) ---
                pz4 = ps_z.tile([128, 512], f32, name="z_ps")
                for ci in range(4):
                    # each column-group starts its own accumulation group on
                    # its own 32 partitions; the sim's group checker doesn't
                    # understand partition-offset groups, so skip it.
                    nc.tensor.matmul(pz4[32 * ci:32 * (ci + 1), :],
                                     identity[:B, :B],
                                     zx_t[:, 512 * ci:512 * (ci + 1)],
                                     start=True, stop=(t == 0),
                                     tile_position=(0, 32 * ci),
                                     skip_group_check=True)
                if t > 0:
                    hT_prev = h_seqT[:, :, B * (t - 1):B * t]
                    for kt in range(NKH):
                        for ci in range(4):
                            nc.tensor.matmul(
                                pz4[32 * ci:32 * (ci + 1), :],
                                hT_prev[:, kt, :],
                                w_h_sb[:, kt, 512 * ci:512 * (ci + 1)],
                                start=False, stop=(kt == NKH - 1),
                                tile_position=(0, 32 * ci),
                                skip_group_check=True)
                gates = {}
                for ci in chunk_order:
                    g_sb = pb.tile([B, 512], f32, name=f"gate{ci}", bufs=1)
                    nc.scalar.activation(g_sb[:], pz4[32 * ci:32 * (ci + 1), :],
                                         gate_funcs[ci])
                    gates[ci] = g_sb

                # --- logits matmuls fill the PE while ACT/DVE run the chain ---
                pls = []
                if c_log >= 0:
                    pls.append((2 * s, logits_mms(c_log, 2 * s)))
                    pls.append((2 * s + 1, logits_mms(c_log, 2 * s + 1)))
                # argmax halves of an older m-tile run in DVE slack; emit
                # before the chain so they don't block it in the DVE FIFO
                if c_arg >= 0:
                    emit_argmax_part(c_arg, s)

                # --- c' = sig(f)*c + sig(i)*tanh(g);  h = sig(o)*tanh(c') ---
                t_a = pb.tile([B, H], f32, name="t_a", bufs=1)
                nc.vector.tensor_tensor(out=t_a[:], in0=gates[1][:],
                                        in1=c_prev[:], op=OP.mult)
                t_b = pb.tile([B, H], f32, name="t_b", bufs=1)
                nc.vector.tensor_tensor(out=t_b[:], in0=gates[0][:],
                                        in1=gates[2][:], op=OP.mult)
                nc.vector.tensor_tensor(out=c_new[:], in0=t_a[:], in1=t_b[:],
                                        op=OP.add)
                # tanh/h/transpose pipelined in 128-wide chunks so the next
                # step's first matmuls can begin before the whole h is done
                t_th = pb.tile([B, H], f32, name="t_th", bufs=1)
                h_sb = pb.tile([B, H], f32, name="h_sb", bufs=2)
                for kt in range(NKH):
                    hsl = slice(HCH * kt, HCH * (kt + 1))
                    nc.scalar.activation(t_th[:, hsl], c_new[:, hsl], AF.Tanh)
                    nc.vector.tensor_tensor(out=h_sb[:, hsl],
                                            in0=gates[3][:, hsl],
                                            in1=t_th[:, hsl], op=OP.mult)
                    ptr = ps_tr.tile([128, B], f32, name="tr_ps")
                    nc.tensor.transpose(out=ptr[:], in_=h_sb[:, hsl],
                                        identity=identity[:B, :B])
                    nc.vector.tensor_copy(h_seqT[:, kt, B * t:B * (t + 1)],
                                          ptr[:])

                # --- evictions late (ACT slack), argmax parts (DVE slack) ---
                for vc, pl in pls:
                    logits_evict(c_log, vc, pl)

            # tail: last m-tile + remaining argmax parts
            for vc in range(len(VCH)):
                logits_evict(NMT - 1, vc, logits_mms(NMT - 1, vc))
            for s in range(4):
                emit_argmax_part(NMT - 2, s)
            for s in range(4):
                emit_argmax_part(NMT - 1, s)

            aarg_i = persist.tile([128, NMT], i32)
            nc.vector.tensor_copy(aarg_i[:], aarg_sb[:])
            nc.sync.dma_start(out_max[:, :], amax_sb[:])
            nc.sync.dma_start(out_arg[:, :], aarg_i[:])

        persist_cm.__exit__(None, None, None)
        dram_cm.__exit__(None, None, None)

    _split_waits(nc, mybir)
    return nc


def _get_program(with_lstm_bias, with_logit_bias):
    key = (with_lstm_bias, with_logit_bias)
    if key not in _PROGRAM_CACHE:
        _PROGRAM_CACHE[key] = build_program(*key)
    return _PROGRAM_CACHE[key]


# --------------------------------------------------------------------------
# Host wrapper
# --------------------------------------------------------------------------
def make_in_maps(top_k_attributes, mean_image_features, seq_inputs, lengths,
                 word_emb, attr_emb, lstm_kernel, lstm_bias, W_logits, b_logits):
    asnp = lambda x: np.ascontiguousarray(np.asarray(x))
    top_k_attributes = asnp(top_k_attributes).astype(np.int32)
    seq_inputs = asnp(seq_inputs).astype(np.int32)
    lengths_np = asnp(lengths).astype(np.int32)
    mean_image_features = asnp(mean_image_features).astype(np.float32)
    word_emb = asnp(word_emb).astype(np.float32)
    attr_emb = asnp(attr_emb).astype(np.float32)
    lstm_kernel = asnp(lstm_kernel).astype(np.float32)
    lstm_bias = asnp(lstm_bias).astype(np.float32)
    W_logits = asnp(W_logits).astype(np.float32)
    b_logits = asnp(b_logits).astype(np.float32)

    w_e = np.ascontiguousarray(lstm_kernel[:DE])
    w_f = np.ascontiguousarray(lstm_kernel[DE:DE + DI + DA])
    w_h = np.ascontiguousarray(lstm_kernel[DE + DI + DA:])

    in_maps = []
    for j in range(NCORE):
        bs = slice(BL * j, BL * (j + 1))
        vs = slice(VS * j, VS * (j + 1))
        seq_j = seq_inputs[bs]                       # [BL, T]
        in_maps.append({
            "seq_tm": np.ascontiguousarray(seq_j.T.reshape(-1)),   # t-major
            "attr_ids": np.ascontiguousarray(top_k_attributes[bs].reshape(-1)),
            "img_T": np.ascontiguousarray(mean_image_features[bs].T),
            "lengths": lengths_np,
            "word_emb": word_emb,
            "attr_emb": attr_emb,
            "w_e": w_e,
            "w_f": w_f,
            "w_h": w_h,
            "lstm_b": lstm_bias,
            "w_log": np.ascontiguousarray(W_logits[:, vs]),
            "b_log": np.ascontiguousarray(b_logits[vs]),
        })
    flags = (bool(np.any(lstm_bias)), bool(np.any(b_logits)))
    return in_maps, flags


def combine_outputs(results):
    logits = np.empty((B, T, VOCAB), np.float32)
    vals = np.empty((NCORE, TOKG), np.float32)
    args = np.empty((NCORE, TOKG), np.int64)
    for j, r in enumerate(results):
        lj = r["out_logits"].reshape(T, B, VS)        # rows are t*32+b
        logits[:, :, VS * j:VS * (j + 1)] = lj.transpose(1, 0, 2)
        vals[j] = r["out_max"].T.reshape(-1)          # token = 128*c + p
        args[j] = r["out_arg"].T.reshape(-1)
    win = np.argmax(vals, axis=0)                     # ties -> lowest core
    tok = np.arange(TOKG)
    gl_arg = (args[win, tok] + VS * win).astype(np.int32)
    preds = gl_arg.reshape(T, B).T.copy()             # [B, T]
    return logits, preds


def kernel(**inputs):
    from concourse.bass_utils import run_bass_kernel_spmd
    in_maps, flags = make_in_maps(**inputs)
    nc = _get_program(*flags)
    res = run_bass_kernel_spmd(nc, in_maps, list(range(NCORE)))
    return combine_outputs(res.results)


# expose for test.py profiling
def kernel_traced(**inputs):
    from concourse.bass_utils import run_bass_kernel_spmd
    in_maps, flags = make_in_maps(**inputs)
    nc = _get_program(*flags)
    res = run_bass_kernel_spmd(nc, in_maps, list(range(NCORE)), trace=True)
    return combine_outputs(res.results), res


# revision 23
# speedup vs baseline: 1.1019x; 1.0137x over previous
"""AttributeImageCaptioner fused kernel for 8 trn2 NeuronCores.

Model (see reference):
  attr/word embedding gathers -> per-step LSTM (T=64, B=32, H=512) over
  inp = [word_emb(300) | image_feats(2048) | attr_mean(300)] -> masked h_seq
  -> vocab projection [512, 32000] (+argmax).

Distribution strategy:
  * The non-recurrent 84% of the LSTM contraction (inp @ W[:2648]) is
    batch-sharded: each core computes Zx for its 4 batch rows, then an
    AllGather shares the full Zx [2048 tok, 2048 gates].
  * The recurrence (h @ W[2648:3160] + gates) is replicated on all cores
    (per-step collectives are slower than the 3.4us/step of replicated work).
  * The vocab projection + argmax is vocab-sharded: each core holds a
    [512, 4000] slice of W_logits in SBUF and writes 1/8 of the logits.
  * Host combines per-core (max, argmax) candidates and concatenates logits.

Layout notes: everything downstream of Zx lives in "transposed" form
(h^T [512, 32] per step) so the recurrent matmul streams W_h as the moving
operand (2.4GHz) with h^T as cheap stationary tiles, and so h_seq^T tiles are
directly the stationary operand of the vocab projection.
"""

import numpy as np

B, T, K = 32, 64, 5
VOCAB, DE, DA, DI, H = 32000, 300, 300, 2048, 512
NCORE = 8
BL = B // NCORE          # batch rows per core
TOK = BL * T             # local tokens per core
TOKG = B * T             # global tokens
VS = VOCAB // NCORE      # vocab slice per core
H4 = 4 * H               # gate width
KX = DE + DI + DA        # non-recurrent contraction (2648)
NMT = TOKG // 128        # logits m-tiles (16)
STEPS_PER_MT = 128 // B  # 4 lstm steps per logits m-tile

_PROGRAM_CACHE = {}


# --------------------------------------------------------------------------
# Wait-split workaround: this walrus build accepts only one semaphore wait
# per CTRL instruction; hoist excess waits onto preceding no-ops.
# --------------------------------------------------------------------------
def _split_waits(nc, mybir, maxw=1):
    ctr = 0
    for f in nc.m.functions:
        for bb in f.blocks:
            new_insts = []
            for inst in bb.instructions:
                si = inst.sync_info
                if si is not None and si.on_wait and len(si.on_wait) > maxw:
                    waits = list(si.on_wait)
                    pre, keep = waits[:-maxw], waits[-maxw:]
                    for i in range(0, len(pre), maxw):
                        ctr += 1
                        nop = mybir.InstNoOp(
                            name=f"I-waitsplit-{ctr}",
                            engine=inst.engine,
                            ins=[],
                            outs=[],
                            sync_info=mybir.SyncInfo(
                                on_wait=list(pre[i:i + maxw]), on_update=[]),
                            text_hint="waitsplit",
                        )
                        new_insts.append(nop)
                        nc.register_instruction(nop, overwrite=True)
                    si.on_wait = keep
                new_insts.append(inst)
            bb.instructions = new_insts


# --------------------------------------------------------------------------
# Program builder
# --------------------------------------------------------------------------
def build_program(with_lstm_bias, with_logit_bias):
    import concourse.bass as bass
    import concourse.mybir as mybir
    import concourse.tile as tile
    from concourse.masks import make_identity

    f32 = mybir.dt.float32
    i32 = mybir.dt.int32
    AF = mybir.ActivationFunctionType
    OP = mybir.AluOpType

    nc = bass.Bass(num_devices=NCORE)

    # ---------------- I/O ----------------
    seq_tm = nc.declare_dram_parameter("seq_tm", [TOK], i32, isOutput=False)
    attr_ids = nc.declare_dram_parameter("attr_ids", [BL * K], i32, isOutput=False)
    img_T = nc.declare_dram_parameter("img_T", [DI, BL], f32, isOutput=False)
    lengths = nc.declare_dram_parameter("lengths", [B], i32, isOutput=False)
    word_emb = nc.declare_dram_parameter("word_emb", [VOCAB, DE], f32, isOutput=False)
    attr_emb = nc.declare_dram_parameter("attr_emb", [1000, DA], f32, isOutput=False)
    w_e = nc.declare_dram_parameter("w_e", [DE, H4], f32, isOutput=False)
    w_f = nc.declare_dram_parameter("w_f", [DI + DA, H4], f32, isOutput=False)
    w_h = nc.declare_dram_parameter("w_h", [H, H4], f32, isOutput=False)
    lstm_b = nc.declare_dram_parameter("lstm_b", [H4], f32, isOutput=False)
    w_log = nc.declare_dram_parameter("w_log", [H, VS], f32, isOutput=False)
    b_log = nc.declare_dram_parameter("b_log", [VS], f32, isOutput=False)

    out_logits = nc.declare_dram_parameter("out_logits", [TOKG, VS], f32, isOutput=True)
    out_max = nc.declare_dram_parameter("out_max", [128, NMT], f32, isOutput=True)
    out_arg = nc.declare_dram_parameter("out_arg", [128, NMT], i32, isOutput=True)

    # ---------------- constants (NEFF-embedded) ----------------
    # t index of token (p, c):  token = 128*c + p  ->  t = 4*c + p//32
    t_idx_np = np.empty((128, NMT), np.float32)
    for c in range(NMT):
        for p in range(128):
            t_idx_np[p, c] = 4 * c + p // 32
    t_idx_dram = nc.inline_tensor(t_idx_np, name="t_idx_const")
    # S^T[b, p] = 1 if p % BL == b : broadcasts per-batch G rows to token rows
    s_np = np.zeros((BL, 128), np.float32)
    for p in range(128):
        s_np[p % BL, p] = 1.0
    s_dram = nc.inline_tensor(s_np, name="s_const")
    # attr mean matrix: M[r, b] = 1/K if r // K == b  (r in [0, BL*K))
    m_np = np.zeros((BL * K, BL), np.float32)
    for r in range(BL * K):
        m_np[r, r // K] = 1.0 / K
    m_dram = nc.inline_tensor(m_np, name="m_const")
    ones_np = np.ones((1, 128), np.float32)
    ones_dram = nc.inline_tensor(ones_np, name="ones_const")

    NWE = 3                      # W_e k-tiles: 128,128,44
    WE_SZ = [128, 128, DE - 256]
    NWF = (DI + DA + 127) // 128  # 19 k-tiles of W_f (img 16 + attr 2.34)
    NKH = H // 128               # 4 k-tiles of h
    VCH = [512] * (VS // 512) + ([VS % 512] if VS % 512 else [])  # vocab chunks

    gate_funcs = [AF.Sigmoid, AF.Sigmoid, AF.Tanh, AF.Sigmoid]  # i, f, g, o
    chunk_order = [1, 0, 2, 3]  # process f, i, g, o

    with tile.TileContext(nc) as tc:
        dram_cm = tc.tile_pool(name="dram", bufs=1, space="DRAM")
        dram = dram_cm.__enter__()
        persist_cm = tc.tile_pool(name="persist", bufs=1)
        persist = persist_cm.__enter__()

        # ------------- persistent SBUF -------------
        identity = persist.tile([128, 128], f32)
        make_identity(nc, identity[:])
        w_h_sb = persist.tile([128, NKH, H4], f32)
        nc.sync.dma_start(w_h_sb[:], w_h.rearrange("(k p) n -> p k n", p=128))
        w_log_sb = persist.tile([128, NKH, VS], f32)
        nc.sync.dma_start(w_log_sb[:], w_log.rearrange("(k p) n -> p k n", p=128))
        h_seqT = persist.tile([128, NKH, TOKG], f32)
        lstmb_sb = None
        if with_lstm_bias:
            lstmb_sb = persist.tile([1, H4], f32)
            nc.sync.dma_start(lstmb_sb[:], lstm_b[None, :])
        blog_sb = None
        if with_logit_bias:
            blog_sb = persist.tile([1, VS], f32)
            nc.sync.dma_start(blog_sb[:], b_log[None, :])
        sT_sb = persist.tile([BL, 128], f32)
        nc.sync.dma_start(sT_sb[:], s_dram[:])
        mM_sb = persist.tile([BL * K, BL], f32)
        nc.sync.dma_start(mM_sb[:], m_dram[:])
        ones_sb = persist.tile([1, 128], f32)
        nc.sync.dma_start(ones_sb[:], ones_dram[:])
        mask_sb = persist.tile([128, NMT], f32)   # (t < len) per (p, mtile)
        amax_sb = persist.tile([128, NMT], f32)
        aarg_sb = persist.tile([128, NMT], f32)
        cst = [persist.tile([B, H], f32, name=f"c_state{i}") for i in range(2)]
        nc.vector.memset(cst[0][:], 0.0)

        # mask: t_idx < len  <=>  len > t_idx
        t_idx_sb = persist.tile([128, NMT], f32)
        nc.sync.dma_start(t_idx_sb[:], t_idx_dram[:])
        len_i = persist.tile([128, 1], i32)
        for r in range(128 // B):
            nc.sync.dma_start(len_i[r * B:(r + 1) * B, :], lengths[:, None])
        len_f = persist.tile([128, 1], f32)
        nc.vector.tensor_copy(len_f[:], len_i[:])
        nc.vector.tensor_tensor(
            out=mask_sb[:], in0=len_f[:].to_broadcast([128, NMT]),
            in1=t_idx_sb[:], op=OP.is_gt)

        bias_bc = None
        if with_logit_bias:
            bias_bc = persist.tile([128, VS], f32)

        # dram scratch for the AllGather (split: steps [0,8) first so the
        # recurrence can start while the big gather is still in flight)
        NT_A = 8
        ROWS_A = BL * NT_A
        zx_localA = dram.tile([ROWS_A, H4], f32)
        zx_localB = dram.tile([TOK - ROWS_A, H4], f32)
        zx_allA = dram.tile([NCORE, ROWS_A, H4], f32, addr_space="Shared")
        zx_allB = dram.tile([NCORE, TOK - ROWS_A, H4], f32, addr_space="Shared")

        # ================= Phase A: embeddings + Zx + AllGather =============
        with tc.tile_pool(name="phaseA", bufs=2) as pa, \
                tc.tile_pool(name="phaseA_w", bufs=2) as pa_w, \
                tc.tile_pool(name="phaseA_ps", bufs=2, space="PSUM") as pa_ps, \
                tc.tile_pool(name="phaseA_psg", bufs=1, space="PSUM") as pa_psg:
            ga_sb = pa.tile([BL, H4], f32, bufs=1)   # feats @ W_f (+bias)
            # ---- gathers ----
            attr_idx = pa.tile([BL * K, 1], i32, bufs=1)
            nc.sync.dma_start(attr_idx[:], attr_ids[:, None])
            attr_g = pa.tile([BL * K, DA], f32, bufs=1)
            nc.gpsimd.indirect_dma_start(
                out=attr_g[:], out_offset=None, in_=attr_emb[:, :],
                in_offset=bass.IndirectOffsetOnAxis(ap=attr_idx[:, :1], axis=0))

            widx = [pa.tile([128, 1], i32, name=f"widx{i}", bufs=1) for i in range(2)]
            wg = [pa.tile([128, DE], f32, name=f"wg{i}", bufs=1) for i in range(2)]
            for i in range(2):
                nc.sync.dma_start(widx[i][:], seq_tm[128 * i:128 * (i + 1), None])
                nc.gpsimd.indirect_dma_start(
                    out=wg[i][:], out_offset=None, in_=word_emb[:, :],
                    in_offset=bass.IndirectOffsetOnAxis(ap=widx[i][:, :1], axis=0))

            # ---- feats^T tiles [128, NWF, BL]: img_T then attr_mean^T ----
            featsT = pa.tile([128, NWF, BL], f32, bufs=1)
            nc.sync.dma_start(
                featsT[:, :DI // 128, :],
                img_T.rearrange("(k p) b -> p k b", p=128))
            # attr mean^T via PE: out = attr_g.T @ M  -> [DA, BL]
            for ft in range(3):
                fs = min(128, DA - 128 * ft)
                pm = pa_ps.tile([128, 128], f32, name="tr_ps", tag="tr_ps")
                nc.tensor.matmul(pm[:fs, :BL], attr_g[:, 128 * ft:128 * ft + fs],
                                 mM_sb[:], start=True, stop=True)
                nc.vector.tensor_copy(featsT[:fs, DI // 128 + ft, :], pm[:fs, :BL])

            # ---- x_word^T tiles [128, NWE, TOK] via PE transpose ----
            xwT = pa.tile([128, NWE, TOK], f32, bufs=1)
            for i in range(2):
                for ft in range(NWE):
                    fs = WE_SZ[ft]
                    pt = pa_ps.tile([128, 128], f32, name="tr_ps", tag="tr_ps")
                    nc.tensor.transpose(
                        out=pt[:fs, :], in_=wg[i][:, 128 * ft:128 * ft + fs],
                        identity=identity[:])
                    nc.vector.tensor_copy(xwT[:fs, ft, 128 * i:128 * (i + 1)],
                                          pt[:fs, :128])

            # ---- G = feats @ W_f (+ lstm bias), [BL, H4] ----
            gps = pa_psg.tile([BL, H4], f32)
            for kt in range(NWF):
                ks = min(128, DI + DA - 128 * kt)
                wft = pa_w.tile([128, H4], f32, name="wf_t")
                nc.sync.dma_start(wft[:ks, :], w_f[128 * kt:128 * kt + ks, :])
                for nch in range(4):
                    nc.tensor.matmul(
                        gps[:, 512 * nch:512 * (nch + 1)],
                        featsT[:ks, kt, :], wft[:ks, 512 * nch:512 * (nch + 1)],
                        start=(kt == 0), stop=(kt == NWF - 1 and not with_lstm_bias))
            if with_lstm_bias:
                for nch in range(4):
                    nc.tensor.matmul(
                        gps[:, 512 * nch:512 * (nch + 1)],
                        ones_sb[:, :BL], lstmb_sb[:, 512 * nch:512 * (nch + 1)],
                        start=False, stop=True)
            nc.vector.tensor_copy(ga_sb[:], gps[:])

            # ---- Zx local = x_word @ W_e + S@G, write to zx_local ----
            we_sb = pa.tile([128, NWE, H4], f32, bufs=1)
            for ft in range(NWE):
                fs = WE_SZ[ft]
                nc.sync.dma_start(we_sb[:fs, ft, :],
                                  w_e[128 * ft:128 * ft + fs, :])
            for i in range(2):
                for nch in range(4):
                    pz = pa_ps.tile([128, 512], f32, name="zx_ps")
                    for ft in range(NWE):
                        fs = WE_SZ[ft]
                        nc.tensor.matmul(
                            pz[:], xwT[:fs, ft, 128 * i:128 * (i + 1)],
                            we_sb[:fs, ft, 512 * nch:512 * (nch + 1)],
                            start=(ft == 0), stop=False)
                    nc.tensor.matmul(
                        pz[:], sT_sb[:], ga_sb[:, 512 * nch:512 * (nch + 1)],
                        start=False, stop=True)
                    zx_sb = pa.tile([128, 512], f32, name="zx_sb", bufs=3)
                    nc.vector.tensor_copy(zx_sb[:], pz[:])
                    sl_v = slice(512 * nch, 512 * (nch + 1))
                    if i == 0:
                        nc.sync.dma_start(zx_localA[:, sl_v], zx_sb[:ROWS_A, :])
                        nc.sync.dma_start(
                            zx_localB[:128 - ROWS_A, sl_v], zx_sb[ROWS_A:, :])
                    else:
                        nc.sync.dma_start(
                            zx_localB[128 - ROWS_A:256 - ROWS_A, sl_v], zx_sb[:])

            nc.gpsimd.collective_compute(
                "AllGather", OP.bypass,
                replica_groups=[list(range(NCORE))],
                ins=[zx_localA[:, :].opt()],
                outs=[zx_allA[:, :, :].opt()])
            nc.gpsimd.collective_compute(
                "AllGather", OP.bypass,
                replica_groups=[list(range(NCORE))],
                ins=[zx_localB[:, :].opt()],
                outs=[zx_allB[:, :, :].opt()])

        # ================= Phase B: recurrence + logits =====================
        with tc.tile_pool(name="phaseB", bufs=2) as pb, \
                tc.tile_pool(name="phaseB_zx", bufs=3) as pb_zx, \
                tc.tile_pool(name="phaseB_lsb", bufs=2) as pb_lsb, \
                tc.tile_pool(name="ps_z", bufs=2, space="PSUM") as ps_z, \
                tc.tile_pool(name="ps_tr", bufs=2, space="PSUM") as ps_tr, \
                tc.tile_pool(name="ps_l", bufs=4, space="PSUM") as ps_l:
            if with_logit_bias:
                for vc, vs in enumerate(VCH):
                    pbias = ps_l.tile([128, 512], f32, name="pl")
                    nc.tensor.matmul(pbias[:, :vs], ones_sb[:],
                                     blog_sb[:, 512 * vc:512 * vc + vs],
                                     start=True, stop=True)
                    nc.vector.tensor_copy(bias_bc[:, 512 * vc:512 * vc + vs],
                                          pbias[:, :vs])

            lsb_by_mt = {}
            argmax_state = {}

            def logits_mms(c_mt, vc):
                """PE part of one [128 tok, <=512 vocab] logits chunk."""
                vs = VCH[vc]
                msl = slice(128 * c_mt, 128 * (c_mt + 1))
                if c_mt not in lsb_by_mt:
                    lsb_by_mt[c_mt] = pb_lsb.tile([128, VS], f32, name="lsb")
                pl = ps_l.tile([128, 512], f32, name="pl")
                for kt in range(NKH):
                    nc.tensor.matmul(
                        pl[:, :vs], h_seqT[:, kt, msl],
                        w_log_sb[:, kt, 512 * vc:512 * vc + vs],
                        start=(kt == 0), stop=(kt == NKH - 1))
                return pl

            def logits_evict(c_mt, vc, pl):
                """Masked eviction (zero rows past the length) + store."""
                vs = VCH[vc]
                msl = slice(128 * c_mt, 128 * (c_mt + 1))
                lsb = lsb_by_mt[c_mt]
                nc.scalar.activation(
                    lsb[:, 512 * vc:512 * vc + vs], pl[:, :vs],
                    AF.Copy, scale=mask_sb[:, c_mt:c_mt + 1])
                if with_logit_bias:
                    nc.vector.tensor_tensor(
                        out=lsb[:, 512 * vc:512 * vc + vs],
                        in0=lsb[:, 512 * vc:512 * vc + vs],
                        in1=bias_bc[:, 512 * vc:512 * vc + vs],
                        op=OP.add)
                nc.sync.dma_start(
                    out_logits[msl, 512 * vc:512 * vc + vs],
                    lsb[:, 512 * vc:512 * vc + vs])

            HALF = VS // 2

            def emit_argmax_part(c_mt, s):
                """Argmax over vocab-halves, spread over 4 steps' DVE slack."""
                lsb = lsb_by_mt[c_mt]
                if s == 0:
                    st = argmax_state[c_mt] = {
                        "mx": pb.tile([128, 2, 8], f32, name="mx8", bufs=2),
                        "ix": pb.tile([128, 2, 8], mybir.dt.uint32, name="ix8",
                                      bufs=2),
                    }
                    nc.vector.max(out=st["mx"][:, 0, :], in_=lsb[:, :HALF])
                elif s == 1:
                    st = argmax_state[c_mt]
                    nc.vector.max(out=st["mx"][:, 1, :], in_=lsb[:, HALF:])
                elif s == 2:
                    st = argmax_state[c_mt]
                    nc.vector.max_index(out=st["ix"][:, 0, :],
                                        in_max=st["mx"][:, 0, :],
                                        in_values=lsb[:, :HALF])
                else:
                    st = argmax_state.pop(c_mt)
                    nc.vector.max_index(out=st["ix"][:, 1, :],
                                        in_max=st["mx"][:, 1, :],
                                        in_values=lsb[:, HALF:])
                    lsb_by_mt.pop(c_mt)
                    # merge halves: prefer the low half on ties (jnp.argmax)
                    vlo, vhi = st["mx"][:, 0, :1], st["mx"][:, 1, :1]
                    sel = pb.tile([128, 1], f32, name="sel", bufs=2)
                    nc.vector.tensor_tensor(out=sel[:], in0=vhi, in1=vlo,
                                            op=OP.is_gt)
                    nc.vector.tensor_tensor(out=amax_sb[:, c_mt:c_mt + 1],
                                            in0=vlo, in1=vhi, op=OP.max)
                    ilo = pb.tile([128, 1], f32, name="ilo", bufs=2)
                    nc.vector.tensor_copy(ilo[:], st["ix"][:, 0, :1])
                    ihi = pb.tile([128, 1], f32, name="ihi", bufs=2)
                    nc.vector.tensor_copy(ihi[:], st["ix"][:, 1, :1])
                    nc.vector.tensor_scalar(ihi[:], ihi[:], float(HALF), None,
                                            op0=OP.add)
                    ixf = pb.tile([128, 1], f32, name="ixf", bufs=2)
                    # ixf = sel ? ihi : ilo  =  ilo + sel*(ihi-ilo)
                    nc.vector.tensor_tensor(out=ixf[:], in0=ihi, in1=ilo,
                                            op=OP.subtract)
                    nc.vector.tensor_tensor(out=ixf[:], in0=ixf[:], in1=sel[:],
                                            op=OP.mult)
                    nc.vector.tensor_tensor(out=ixf[:], in0=ixf[:], in1=ilo[:],
                                            op=OP.add)
                    if not with_logit_bias:
                        # all-masked rows are all-zero; force argmax to 0 to
                        # match jnp.argmax's lowest-index tie-break
                        nc.vector.tensor_tensor(
                            out=ixf[:], in0=ixf[:],
                            in1=mask_sb[:, c_mt:c_mt + 1], op=OP.mult)
                    nc.vector.tensor_copy(aarg_sb[:, c_mt:c_mt + 1], ixf[:])

            HCH = 128                      # h is produced in 128-col chunks
            for t in range(T):
                c_prev, c_new = cst[t % 2], cst[(t + 1) % 2]
                s = t % STEPS_PER_MT
                c_log = t // STEPS_PER_MT - 1      # m-tile getting its logits
                c_arg = t // STEPS_PER_MT - 2      # m-tile getting its argmax

                zx_t = pb_zx.tile([B, H4], f32, name="zx_t")
                if t < NT_A:
                    nc.sync.dma_start(zx_t[:], zx_allA[:, BL * t:BL * (t + 1), :])
                else:
                    tb = t - NT_A
                    nc.sync.dma_start(zx_t[:], zx_allB[:, BL * tb:BL * (tb + 1), :])

                # --- recurrent matmuls: the four 512-wide gate chunks run
                # CONCURRENTLY in the four 32-column groups of the PE array
                # (M=32 per chunk; tile_position col-tiling) ---
                pz4 = ps_z.tile([128, 512], f32, name="z_ps")
                for ci in range(4):
                    # each column-group starts its own accumulation group on
                    # its own 32 partitions; the sim's group checker doesn't
                    # understand partition-offset groups, so skip it.
                    nc.tensor.matmul(pz4[32 * ci:32 * (ci + 1), :],
                                     identity[:B, :B],
                                     zx_t[:, 512 * ci:512 * (ci + 1)],
                                     start=True, stop=(t == 0),
                                     tile_position=(0, 32 * ci),
                                     skip_group_check=True)
                if t > 0:
                    hT_prev = h_seqT[:, :, B * (t - 1):B * t]
                    for kt in range(NKH):
                        for ci in range(4):
                            nc.tensor.matmul(
                                pz4[32 * ci:32 * (ci + 1), :],
                                hT_prev[:, kt, :],
                                w_h_sb[:, kt, 512 * ci:512 * (ci + 1)],
                                start=False, stop=(kt == NKH - 1),
                                tile_position=(0, 32 * ci),
                                skip_group_check=True)
                gates = {}
                for ci in chunk_order:
                    g_sb = pb.tile([B, 512], f32, name=f"gate{ci}", bufs=1)
                    if ci == 3:
                        nc.scalar.activation(g_sb[:],
                                             pz4[32 * ci:32 * (ci + 1), :],
                                             gate_funcs[ci])
                    else:
                        # halves: lets the DVE chain start ~2 ACT-ops earlier
                        for hh in range(2):
                            s2 = slice(256 * hh, 256 * (hh + 1))
                            nc.scalar.activation(
                                g_sb[:, s2], pz4[32 * ci:32 * (ci + 1), s2],
                                gate_funcs[ci])
                    gates[ci] = g_sb

                # --- logits matmuls fill the PE while ACT/DVE run the chain ---
                pls = []
                if c_log >= 0:
                    pls.append((2 * s, logits_mms(c_log, 2 * s)))
                    pls.append((2 * s + 1, logits_mms(c_log, 2 * s + 1)))
                # argmax halves of an older m-tile run in DVE slack; emit
                # before the chain so they don't block it in the DVE FIFO
                if c_arg >= 0:
                    emit_argmax_part(c_arg, s)

                # --- c' = sig(f)*c + sig(i)*tanh(g);  h = sig(o)*tanh(c') ---
                t_a = pb.tile([B, H], f32, name="t_a", bufs=1)
                t_b = pb.tile([B, H], f32, name="t_b", bufs=1)
                for hh in range(2):
                    s2 = slice(256 * hh, 256 * (hh + 1))
                    nc.vector.tensor_tensor(out=t_a[:, s2], in0=gates[1][:, s2],
                                            in1=c_prev[:, s2], op=OP.mult)
                    nc.vector.tensor_tensor(out=t_b[:, s2], in0=gates[0][:, s2],
                                            in1=gates[2][:, s2], op=OP.mult)
                    nc.vector.tensor_tensor(out=c_new[:, s2], in0=t_a[:, s2],
                                            in1=t_b[:, s2], op=OP.add)
                # tanh/h/transpose pipelined in 128-wide chunks so the next
                # step's first matmuls can begin before the whole h is done
                t_th = pb.tile([B, H], f32, name="t_th", bufs=1)
                h_sb = pb.tile([B, H], f32, name="h_sb", bufs=2)
                for kt in range(NKH):
                    hsl = slice(HCH * kt, HCH * (kt + 1))
                    nc.scalar.activation(t_th[:, hsl], c_new[:, hsl], AF.Tanh)
                    nc.vector.tensor_tensor(out=h_sb[:, hsl],
                                            in0=gates[3][:, hsl],
                                            in1=t_th[:, hsl], op=OP.mult)
                    ptr = ps_tr.tile([128, B], f32, name="tr_ps")
                    nc.tensor.transpose(out=ptr[:], in_=h_sb[:, hsl],
                                        identity=identity[:B, :B])
                    nc.vector.tensor_copy(h_seqT[:, kt, B * t:B * (t + 1)],
                                          ptr[:])

                # --- evictions late (ACT slack), argmax parts (DVE slack) ---
                for vc, pl in pls:
                    logits_evict(c_log, vc, pl)

            # tail: last m-tile + remaining argmax parts
            for vc in range(len(VCH)):
                logits_evict(NMT - 1, vc, logits_mms(NMT - 1, vc))
            for s in range(4):
                emit_argmax_part(NMT - 2, s)
            for s in range(4):
                emit_argmax_part(NMT - 1, s)

            aarg_i = persist.tile([128, NMT], i32)
            nc.vector.tensor_copy(aarg_i[:], aarg_sb[:])
            nc.sync.dma_start(out_max[:, :], amax_sb[:])
            nc.sync.dma_start(out_arg[:, :], aarg_i[:])

        persist_cm.__exit__(None, None, None)
        dram_cm.__exit__(None, None, None)

    _split_waits(nc, mybir)
    return nc


def _get_program(with_lstm_bias, with_logit_bias):
    key = (with_lstm_bias, with_logit_bias)
    if key not in _PROGRAM_CACHE:
        _PROGRAM_CACHE[key] = build_program(*key)
    return _PROGRAM_CACHE[key]


# --------------------------------------------------------------------------
# Host wrapper
# --------------------------------------------------------------------------
def make_in_maps(top_k_attributes, mean_image_features, seq_inputs, lengths,
                 word_emb, attr_emb, lstm_kernel, lstm_bias, W_logits, b_logits):
    asnp = lambda x: np.ascontiguousarray(np.asarray(x))
    top_k_attributes = asnp(top_k_attributes).astype(np.int32)
    seq_inputs = asnp(seq_inputs).astype(np.int32)
    lengths_np = asnp(lengths).astype(np.int32)
    mean_image_features = asnp(mean_image_features).astype(np.float32)
    word_emb = asnp(word_emb).astype(np.float32)
    attr_emb = asnp(attr_emb).astype(np.float32)
    lstm_kernel = asnp(lstm_kernel).astype(np.float32)
    lstm_bias = asnp(lstm_bias).astype(np.float32)
    W_logits = asnp(W_logits).astype(np.float32)
    b_logits = asnp(b_logits).astype(np.float32)

    w_e = np.ascontiguousarray(lstm_kernel[:DE])
    w_f = np.ascontiguousarray(lstm_kernel[DE:DE + DI + DA])
    w_h = np.ascontiguousarray(lstm_kernel[DE + DI + DA:])

    in_maps = []
    for j in range(NCORE):
        bs = slice(BL * j, BL * (j + 1))
        vs = slice(VS * j, VS * (j + 1))
        seq_j = seq_inputs[bs]                       # [BL, T]
        in_maps.append({
            "seq_tm": np.ascontiguousarray(seq_j.T.reshape(-1)),   # t-major
            "attr_ids": np.ascontiguousarray(top_k_attributes[bs].reshape(-1)),
            "img_T": np.ascontiguousarray(mean_image_features[bs].T),
            "lengths": lengths_np,
            "word_emb": word_emb,
            "attr_emb": attr_emb,
            "w_e": w_e,
            "w_f": w_f,
            "w_h": w_h,
            "lstm_b": lstm_bias,
            "w_log": np.ascontiguousarray(W_logits[:, vs]),
            "b_log": np.ascontiguousarray(b_logits[vs]),
        })
    flags = (bool(np.any(lstm_bias)), bool(np.any(b_logits)))
    return in_maps, flags


def combine_outputs(results):
    logits = np.empty((B, T, VOCAB), np.float32)
    vals = np.empty((NCORE, TOKG), np.float32)
    args = np.empty((NCORE, TOKG), np.int64)
    for j, r in enumerate(results):
        lj = r["out_logits"].reshape(T, B, VS)        # rows are t*32+b
        logits[:, :, VS * j:VS * (j + 1)] = lj.transpose(1, 0, 2)
        vals[j] = r["out_max"].T.reshape(-1)          # token = 128*c + p
        args[j] = r["out_arg"].T.reshape(-1)
    win = np.argmax(vals, axis=0)                     # ties -> lowest core
    tok = np.arange(TOKG)
    gl_arg = (args[win, tok] + VS * win).astype(np.int32)
    preds = gl_arg.reshape(T, B).T.copy()             # [B, T]
    return logits, preds


def kernel(**inputs):
    from concourse.bass_utils import run_bass_kernel_spmd
    in_maps, flags = make_in_maps(**inputs)
    nc = _get_program(*flags)
    res = run_bass_kernel_spmd(nc, in_maps, list(range(NCORE)))
    return combine_outputs(res.results)


# expose for test.py profiling
def kernel_traced(**inputs):
    from concourse.bass_utils import run_bass_kernel_spmd
    in_maps, flags = make_in_maps(**inputs)
    nc = _get_program(*flags)
    res = run_bass_kernel_spmd(nc, in_maps, list(range(NCORE)), trace=True)
    return combine_outputs(res.results), res
